# revision 1
# baseline (speedup 1.0000x reference)
"""Trainium2 Bass kernel for InteractorwoLSTM additive attention.

out[b,t,:] = alpha[b,t,:] @ h_s[b]  with
  beta[b,t,n] = W_w . tanh(h_s[b,n]@W_S + b_S + h_v[b,t]@W_V + b_V) + b_w
  alpha = masked-softmax(beta) per reference semantics.

Sharding: data-parallel over batch B=32 across 8 cores (4 batches/core);
all weights replicated.

Device layout (per core, per batch b):
  - D_I (=512) lives on partitions in 4 chunks of 128.
  - VT[c]  = (V[b]).T chunk      (128 d, 128 t)   via PE transpose + matmul
  - ST'[c] = (S[b]).T chunk + (b_S+b_V)  (128 d, 30 n)
  - e_pre  = VT broadcast-add ST'  (128, 30, 128)  on DVE (0-stride APs)
  - e      = tanh(e_pre)           on ACT
  - beta   = per-n matmuls lhsT=e[:,n,:], rhs=W_w chunk -> psum (128 t, 30 n)
  - masked softmax fused on DVE/ACT (exp accum_out gives Z; ttr gives Qsum)
  - alpha^T via PE transpose, final einsum = one matmul (K=30, N=512)
"""

import os
import numpy as np

B, T, N = 32, 128, 30
D = 512
NCORES = 8
BPC = B // NCORES  # batches per core
NC_CHUNKS = D // 128  # 4

_CACHE = {}


def _build(e_dtype_name: str, add_mode: str):
    import concourse.bacc as bacc
    import concourse.tile as tile
    from concourse import mybir
    import concourse.bass as bass
    from concourse.masks import make_identity

    f32 = mybir.dt.float32
    DT_E = getattr(mybir.dt, e_dtype_name)
    DT_VS = DT_E  # dtype of VT/ST tiles (bf16 enables DVE 4x tensor_scalar)

    nc = bacc.Bacc(
        "TRN2",
        target_bir_lowering=False,
        debug=False,
        enable_asserts=True,
        num_devices=NCORES,
    )

    # ---- DRAM I/O ----
    hs_d = nc.dram_tensor("h_s", [BPC, N, D], f32, kind="ExternalInput").ap()
    hv_d = nc.dram_tensor("h_v", [BPC, T, D], f32, kind="ExternalInput").ap()
    WS_d = nc.dram_tensor("W_S", [D, D], f32, kind="ExternalInput").ap()
    WV_d = nc.dram_tensor("W_V", [D, D], f32, kind="ExternalInput").ap()
    Ww_d = nc.dram_tensor("W_w", [D], f32, kind="ExternalInput").ap()
    bSV_d = nc.dram_tensor("bSV", [1, D], f32, kind="ExternalInput").ap()
    bw_d = nc.dram_tensor("b_w_rep", [128, 1], f32, kind="ExternalInput").ap()
    mask_d = nc.dram_tensor("mask_bc", [128, BPC, N], f32, kind="ExternalInput").ap()
    out_d = nc.dram_tensor("out", [BPC, T, D], f32, kind="ExternalOutput").ap()

    with tile.TileContext(nc) as tc:
        with (
            tc.tile_pool(name="const", bufs=1) as const,
            tc.tile_pool(name="hv", bufs=2) as hvp,
            tc.tile_pool(name="proj", bufs=2) as projp,
            tc.tile_pool(name="epre", bufs=2) as eprep,
            tc.tile_pool(name="ebig", bufs=2 if DT_E != f32 else 1) as ebigp,
            tc.tile_pool(name="soft", bufs=2) as softp,
            tc.tile_pool(name="pwork", bufs=3, space="PSUM") as pwork,
            tc.tile_pool(name="pbeta", bufs=2, space="PSUM") as pbeta,
            tc.tile_pool(name="pfin", bufs=2, space="PSUM") as pfin,
        ):
            # ---- constants / weights ----
            ident = const.tile([128, 128], f32)
            make_identity(nc, ident[:])

            WS_sb = const.tile([128, NC_CHUNKS, NC_CHUNKS, 128], f32)
            nc.sync.dma_start(
                out=WS_sb[:],
                in_=WS_d.rearrange("(kc p) (mc m) -> p kc mc m", p=128, m=128),
            )
            WV_sb = const.tile([128, NC_CHUNKS, NC_CHUNKS, 128], f32)
            nc.sync.dma_start(
                out=WV_sb[:],
                in_=WV_d.rearrange("(kc p) (mc m) -> p kc mc m", p=128, m=128),
            )
            Ww_sb = const.tile([128, NC_CHUNKS], DT_E)
            nc.sync.dma_start(out=Ww_sb[:], in_=Ww_d.rearrange("(c p) -> p c", p=128))
            bSV_sb = const.tile([1, D], f32)
            nc.sync.dma_start(out=bSV_sb[:], in_=bSV_d)
            bw_sb = const.tile([128, 1], f32)
            nc.sync.dma_start(out=bw_sb[:], in_=bw_d)
            mask_sb = const.tile([128, BPC, N], f32)
            nc.sync.dma_start(out=mask_sb[:], in_=mask_d)
            ones30 = const.tile([1, N], f32)
            nc.vector.memset(ones30[:], 1.0)
            hs_sb = const.tile([N, BPC, D], f32)
            for b in range(BPC):
                nc.sync.dma_start(out=hs_sb[:, b, :], in_=hs_d[b])

            for b in range(BPC):
                # ---- load + transpose h_v[b]; transpose h_s[b] ----
                hv_sb = hvp.tile([128, D], f32, tag="hv")
                nc.sync.dma_start(out=hv_sb[:], in_=hv_d[b])
                hvT = projp.tile([128, NC_CHUNKS, 128], f32, tag="hvT")
                hsT = projp.tile([128, NC_CHUNKS, N], f32, tag="hsT")
                for c in range(NC_CHUNKS):
                    ps = pwork.tile([128, 128], f32, tag="w")
                    nc.tensor.transpose(
                        ps[:, :128], hv_sb[:, c * 128 : (c + 1) * 128], ident[:]
                    )
                    nc.vector.tensor_copy(hvT[:, c, :], ps[:, :128])
                for c in range(NC_CHUNKS):
                    ps = pwork.tile([128, 128], f32, tag="w")
                    nc.tensor.transpose(
                        ps[:, :N],
                        hs_sb[:, b, c * 128 : (c + 1) * 128],
                        ident[:N, :N],
                    )
                    nc.vector.tensor_copy(hsT[:, c, :], ps[:, :N])

                # ---- projections: VT = (h_v W_V).T, ST' = (h_s W_S).T + bSV ----
                VT = projp.tile([128, NC_CHUNKS, 128], DT_VS, tag="VT")
                ST = projp.tile([128, NC_CHUNKS, N], DT_VS, tag="ST")
                for mc in range(NC_CHUNKS):
                    ps = pwork.tile([128, 128], f32, tag="w")
                    for kc in range(NC_CHUNKS):
                        nc.tensor.matmul(
                            ps[:, :128],
                            WV_sb[:, kc, mc, :],
                            hvT[:, kc, :],
                            start=(kc == 0),
                            stop=(kc == NC_CHUNKS - 1),
                        )
                    nc.vector.tensor_copy(VT[:, mc, :], ps[:, :128])
                for mc in range(NC_CHUNKS):
                    ps = pwork.tile([128, 128], f32, tag="w")
                    for kc in range(NC_CHUNKS):
                        nc.tensor.matmul(
                            ps[:, :N],
                            WS_sb[:, kc, mc, :],
                            hsT[:, kc, :],
                            start=(kc == 0),
                            stop=False,
                        )
                    nc.tensor.matmul(
                        ps[:, :N],
                        bSV_sb[0:1, mc * 128 : (mc + 1) * 128],
                        ones30[0:1, :],
                        start=False,
                        stop=True,
                    )
                    nc.vector.tensor_copy(ST[:, mc, :], ps[:, :N])

                # ---- e = tanh(VT (+bcast) ST') ; beta accumulation ----
                ebig = ebigp.tile([128, NC_CHUNKS, N, 128], DT_E, tag="e")
                beta_ps = pbeta.tile([128, N], f32, tag="beta")
                for c in range(NC_CHUNKS):
                    epre = eprep.tile([128, N, 128], DT_E, tag="epre")
                    if add_mode == "tt":
                        vt_b = VT[:, c, :].unsqueeze(1).broadcast_to([128, N, 128])
                        st_b = ST[:, c, :].unsqueeze(2).broadcast_to([128, N, 128])
                        nc.vector.tensor_add(epre[:], vt_b, st_b)
                    else:  # "ts": per-n tensor_scalar (per-partition scalar add)
                        for n in range(N):
                            nc.vector.tensor_scalar_add(
                                epre[:, n, :],
                                VT[:, c, :],
                                ST[:, c, n : n + 1],
                            )
                    nc.scalar.activation(
                        ebig[:, c, :, :],
                        epre[:],
                        mybir.ActivationFunctionType.Tanh,
                    )
                for n in range(N):
                    for c in range(NC_CHUNKS):
                        nc.tensor.matmul(
                            beta_ps[:, n : n + 1],
                            ebig[:, c, n, :],
                            Ww_sb[:, c : c + 1],
                            start=(c == 0),
                            stop=(c == NC_CHUNKS - 1),
                        )

                # ---- masked softmax (faithful to reference) ----
                m_b = mask_sb[:, b, :]
                q1 = softp.tile([128, N], f32, tag="q1")
                # q1 = (beta + b_w) * m
                nc.vector.tensor_scalar_add(q1[:], beta_ps[:], bw_sb[:])
                nc.vector.tensor_mul(q1[:], q1[:], m_b)
                t1 = softp.tile([128, N], f32, tag="t1")
                Z1 = softp.tile([128, 1], f32, tag="Z1")
                nc.scalar.activation(
                    t1[:], q1[:], mybir.ActivationFunctionType.Exp, accum_out=Z1[:]
                )
                q = softp.tile([128, N], f32, tag="q")
                Qs = softp.tile([128, 1], f32, tag="Qs")
                nc.vector.tensor_mul(q[:], t1[:], m_b)
                qc = softp.tile([128, N], f32, tag="qc")
                nc.scalar.activation(
                    qc[:], q[:], mybir.ActivationFunctionType.Copy, accum_out=Qs[:]
                )
                denom = softp.tile([128, 1], f32, tag="denom")
                nc.vector.tensor_scalar(
                    denom[:],
                    Z1[:],
                    1e-13,
                    Qs[:],
                    op0=mybir.AluOpType.mult,
                    op1=mybir.AluOpType.add,
                )
                recip = softp.tile([128, 1], f32, tag="recip")
                nc.vector.reciprocal(recip[:], denom[:])
                alpha = softp.tile([128, N], f32, tag="alpha")
                nc.vector.tensor_scalar(
                    alpha[:],
                    q[:],
                    recip[:],
                    1e-13,
                    op0=mybir.AluOpType.mult,
                    op1=mybir.AluOpType.add,
                )

                # ---- out[b] = alpha @ h_s[b] ----
                aT_ps = pfin.tile([N, 128], f32, tag="fin")
                nc.tensor.transpose(aT_ps[:], alpha[:], ident[:])
                aT = softp.tile([N, 128], f32, tag="aT")
                nc.vector.tensor_copy(aT[:], aT_ps[:])
                out_ps = pfin.tile([128, D], f32, tag="fin")
                nc.tensor.matmul(out_ps[:], aT[:], hs_sb[:, b, :], start=True, stop=True)
                out_sb = softp.tile([128, D], f32, tag="out")
                nc.vector.tensor_copy(out_sb[:], out_ps[:])
                nc.sync.dma_start(out=out_d[b], in_=out_sb[:])

    nc.compile()
    return nc


def _get_nc():
    e_dtype = os.environ.get("KERNEL_E_DTYPE", "float32")
    add_mode = os.environ.get("KERNEL_ADD_MODE", "tt")
    key = (e_dtype, add_mode)
    if key not in _CACHE:
        _CACHE[key] = _build(e_dtype, add_mode)
    return _CACHE[key]


def _make_in_maps(h_s, h_v, lengths, W_S, b_S, W_V, b_V, W_w, b_w):
    h_s = np.ascontiguousarray(h_s, dtype=np.float32)
    h_v = np.ascontiguousarray(h_v, dtype=np.float32)
    mask = (
        np.asarray(lengths).reshape(B, 1) >= np.arange(1, N + 1).reshape(1, N)
    ).astype(np.float32)
    WS = np.ascontiguousarray(W_S, dtype=np.float32)
    WV = np.ascontiguousarray(W_V, dtype=np.float32)
    Ww = np.ascontiguousarray(W_w, dtype=np.float32)
    bSV = np.ascontiguousarray((b_S + b_V).reshape(1, D), dtype=np.float32)
    bw_rep = np.full((128, 1), np.float32(np.asarray(b_w).reshape(-1)[0]))
    in_maps = []
    for c in range(NCORES):
        sl = slice(c * BPC, (c + 1) * BPC)
        mask_bc = np.ascontiguousarray(
            np.broadcast_to(mask[sl][None, :, :], (128, BPC, N)), dtype=np.float32
        )
        in_maps.append(
            {
                "h_s": h_s[sl],
                "h_v": h_v[sl],
                "W_S": WS,
                "W_V": WV,
                "W_w": Ww,
                "bSV": bSV,
                "b_w_rep": bw_rep,
                "mask_bc": mask_bc,
            }
        )
    return in_maps


def run(inputs: dict, trace: bool = False):
    """Run on 8 NeuronCores; returns (output, BassKernelResults)."""
    from concourse import bass_utils

    nc = _get_nc()
    in_maps = _make_in_maps(**inputs)
    res = bass_utils.run_bass_kernel_spmd(
        nc, in_maps, core_ids=list(range(NCORES)), trace=trace
    )
    outs = [r["out"] for r in res.results]
    full = np.concatenate(outs, axis=0).astype(np.float32)
    return full, res


def kernel(**inputs) -> np.ndarray:
    out, _ = run(inputs, trace=False)
    return out



# revision 13
# speedup vs baseline: 4.2130x; 4.2130x over previous
"""Trainium2 Bass kernel for InteractorwoLSTM additive attention.

out[b,t,:] = alpha[b,t,:] @ h_s[b]  with
  beta[b,t,n] = W_w . tanh(h_s[b,n]@W_S + b_S + h_v[b,t]@W_V + b_V) + b_w
  alpha = masked-softmax(beta) per reference semantics.

Sharding: data-parallel over batch B=32 across 8 cores (4 slots/core).
Batches are dealt to cores sorted by length (descending) so that slot j
has a compile-time bound NJ[j] = max length within slot j; all work in
the token dimension n is clipped to NJ[j] (masked-softmax semantics make
positions n >= length irrelevant except for an exp(0)=1 term that the
mask columns reproduce exactly).

Per core, per batch b (D=512 split into 4 chunks of 128 on partitions):
  PE  : VT = (h_v W_V)^T, ST = (h_s W_S)^T   (fp16 inputs, fp32 psum)
  DVE/Pool: epre[d,n,t] = ST[d,n] + bSV[d] + VT[d,t]  (one fused op)
  ACT : e = tanh(epre) -> fp16
  PE  : beta[t,n] = sum_d e[d,n,t]*Ww[d]  (per-(n,chunk) matvec, fp16)
  PE  : betaT = transpose(beta)
  ACT : qT[n,t] = exp(betaT*mask[n] + b_w*mask[n]) -> bf16
  PE  : outraw[t,:] = qT^T @ [h_s*mask | mask]  (ones col gives Qs)
  DVE : out = outraw[:, :512] * (1/Qs[t])
"""

import os
import numpy as np
import ml_dtypes

B, T, N = 32, 128, 30
D = 512
NCORES = 8
BPC = B // NCORES  # batches per core (slots)
NC = D // 128  # 4 chunks

_CACHE = {}

F16 = np.float16
BF16 = ml_dtypes.bfloat16


def _build(nj: tuple, f_dve: float, debug: bool = False):
    """Compile the per-core program. nj[j] = token bound for slot j."""
    import concourse.bacc as bacc
    import concourse.tile as tile
    from concourse import mybir
    from concourse.masks import make_identity

    f32 = mybir.dt.float32
    f16 = mybir.dt.float16
    bf16 = mybir.dt.bfloat16
    ADD = mybir.AluOpType.add
    Tanh = mybir.ActivationFunctionType.Tanh
    Exp = mybir.ActivationFunctionType.Exp

    nc = bacc.Bacc(
        "TRN2",
        target_bir_lowering=False,
        debug=False,
        enable_asserts=True,
        num_devices=NCORES,
    )

    # ---- DRAM I/O (all pre-arranged/cast host-side) ----
    hvT_d = nc.dram_tensor("hvT", [BPC, 128, NC, 128], f16, kind="ExternalInput").ap()
    hsT_d = nc.dram_tensor("hsT", [BPC, 128, NC, N], f16, kind="ExternalInput").ap()
    hsa_d = nc.dram_tensor("hs_aug", [BPC, N, D + 1], bf16, kind="ExternalInput").ap()
    WS_d = nc.dram_tensor("WS16", [128, NC, NC, 128], f16, kind="ExternalInput").ap()
    WV_d = nc.dram_tensor("WV16", [128, NC, NC, 128], f16, kind="ExternalInput").ap()
    Ww_d = nc.dram_tensor("Ww16", [128, NC], f16, kind="ExternalInput").ap()
    bSV_d = nc.dram_tensor("bSV", [128, NC], f32, kind="ExternalInput").ap()
    mT_d = nc.dram_tensor("mT", [N, BPC], f32, kind="ExternalInput").ap()
    bwmT_d = nc.dram_tensor("bwmT", [N, BPC], f32, kind="ExternalInput").ap()
    out_d = nc.dram_tensor("out", [BPC, T, D], f32, kind="ExternalOutput").ap()
    if debug:
        dbg_vt = nc.dram_tensor(
            "dbg_vt", [BPC, 128, NC, 128], f32, kind="ExternalOutput"
        ).ap()
        dbg_st = nc.dram_tensor(
            "dbg_st", [BPC, 128, NC, N], f32, kind="ExternalOutput"
        ).ap()
        dbg_beta = nc.dram_tensor(
            "dbg_beta", [BPC, 128, N], f32, kind="ExternalOutput"
        ).ap()
        dbg_q = nc.dram_tensor(
            "dbg_q", [BPC, N, 128], mybir.dt.bfloat16, kind="ExternalOutput"
        ).ap()

    with tile.TileContext(nc) as tc:
        with (
            tc.tile_pool(name="const", bufs=1) as const,
            tc.tile_pool(name="hvt", bufs=2) as hvtp,
            tc.tile_pool(name="hst", bufs=2) as hstp,
            tc.tile_pool(name="proj", bufs=2) as projp,
            tc.tile_pool(name="epre", bufs=2) as eprep,
            tc.tile_pool(name="ebig", bufs=2) as ebigp,
            tc.tile_pool(name="soft", bufs=2) as softp,
            tc.tile_pool(name="outs", bufs=2) as outsp,
            tc.tile_pool(name="pvt", bufs=2, space="PSUM") as pvt,
            tc.tile_pool(name="pst", bufs=1, space="PSUM") as pst,
            tc.tile_pool(name="pbeta", bufs=1, space="PSUM") as pbeta,
            tc.tile_pool(name="pbt", bufs=1, space="PSUM") as pbt,
            tc.tile_pool(name="pout", bufs=2, space="PSUM") as pout,
        ):
            # ---- constants / weights ----
            ident = const.tile([128, 128], f32)
            make_identity(nc, ident[:])
            WS_sb = const.tile([128, NC, NC, 128], f16)
            nc.sync.dma_start(out=WS_sb[:], in_=WS_d)
            WV_sb = const.tile([128, NC, NC, 128], f16)
            nc.sync.dma_start(out=WV_sb[:], in_=WV_d)
            Ww_sb = const.tile([128, NC], f16)
            nc.sync.dma_start(out=Ww_sb[:], in_=Ww_d)
            bSV_sb = const.tile([128, NC], f32)
            nc.sync.dma_start(out=bSV_sb[:], in_=bSV_d)
            mT_sb = const.tile([N, BPC], f32)
            nc.sync.dma_start(out=mT_sb[:], in_=mT_d)
            bwmT_sb = const.tile([N, BPC], f32)
            nc.sync.dma_start(out=bwmT_sb[:], in_=bwmT_d)
            hs_sb = const.tile([N, BPC, D + 1], bf16)
            for b in range(BPC):
                nc.sync.dma_start(out=hs_sb[:, b, :], in_=hsa_d[b])

            for b in range(BPC):
                NJ = nj[b]
                # ---- load pre-transposed activations ----
                hvT = hvtp.tile([128, NC, 128], f16, tag="hvT")
                nc.sync.dma_start(out=hvT[:], in_=hvT_d[b])
                hsT = hstp.tile([128, NC, N], f16, tag="hsT")
                nc.sync.dma_start(out=hsT[:], in_=hsT_d[b])

                # ---- projections: VT = (h_v W_V).T, ST = (h_s W_S).T ----
                ps_v = pvt.tile([128, D], mybir.dt.float32, tag="vt")
                for mc in range(NC):
                    for kc in range(NC):
                        nc.tensor.matmul(
                            ps_v[:, mc * 128 : (mc + 1) * 128],
                            WV_sb[:, kc, mc, :],
                            hvT[:, kc, :],
                            start=(kc == 0),
                            stop=(kc == NC - 1),
                        )
                VT = projp.tile([128, NC, 128], f32, tag="VT")
                nc.vector.tensor_copy(VT[:], ps_v[:])
                ps_s = pst.tile([128, NC, N], mybir.dt.float32, tag="st")
                for mc in range(NC):
                    for kc in range(NC):
                        nc.tensor.matmul(
                            ps_s[:, mc, :NJ],
                            WS_sb[:, kc, mc, :],
                            hsT[:, kc, :NJ],
                            start=(kc == 0),
                            stop=(kc == NC - 1),
                        )
                ST = projp.tile([128, NC, N], f32, tag="ST")
                for mc in range(NC):
                    nc.vector.tensor_scalar_add(
                        ST[:, mc, :NJ], ps_s[:, mc, :NJ], bSV_sb[:, mc : mc + 1]
                    )

                # ---- e = tanh(ST + bSV + VT); DVE/Pool split over n ----
                k = min(NJ, max(1, int(round(NJ * f_dve))))
                ebig = ebigp.tile([128, NC, N, 128], f16, tag="e")
                for c in range(NC):
                    epre = eprep.tile([128, N, 128], f32, tag="epre")
                    vt_b = VT[:, c, :].unsqueeze(1)
                    nc.vector.tensor_add(
                        epre[:, :k, :],
                        ST[:, c, :k].unsqueeze(2).broadcast_to([128, k, 128]),
                        vt_b.broadcast_to([128, k, 128]),
                    )
                    if k < NJ:
                        nc.gpsimd.tensor_add(
                            epre[:, k:NJ, :],
                            ST[:, c, k:NJ].unsqueeze(2).broadcast_to(
                                [128, NJ - k, 128]
                            ),
                            vt_b.broadcast_to([128, NJ - k, 128]),
                        )
                    nc.scalar.activation(
                        ebig[:, c, :NJ, :], epre[:, :NJ, :], Tanh
                    )

                # ---- beta[t,n] = sum_d e[d,n,t] * Ww[d] ----
                bp = pbeta.tile([128, N], mybir.dt.float32, tag="beta")
                for n in range(NJ):
                    for c in range(NC):
                        nc.tensor.matmul(
                            bp[:, n : n + 1],
                            ebig[:, c, n, :],
                            Ww_sb[:, c : c + 1],
                            start=(c == 0),
                            stop=(c == NC - 1),
                        )
                beta_sb = softp.tile([128, N], f32, tag="bsb")
                nc.vector.tensor_copy(beta_sb[:, :NJ], bp[:, :NJ])

                # ---- betaT, fused masked exp ----
                bt = pbt.tile([32, 128], mybir.dt.float32, tag="bt")
                nc.tensor.transpose(bt[:NJ, :], beta_sb[:, :NJ], ident[:])
                qT = softp.tile([N, 128], bf16, tag="qT")
                nc.scalar.activation(
                    qT[:NJ, :],
                    bt[:NJ, :],
                    Exp,
                    bias=bwmT_sb[:NJ, b : b + 1],
                    scale=mT_sb[:NJ, b : b + 1],
                )

                # ---- out = (qT^T @ [hs*m | m]) scaled by 1/Qs ----
                oq = pbt.tile([128, 1], mybir.dt.float32, tag="oq")
                nc.tensor.matmul(
                    oq[:], qT[:NJ, :], hs_sb[:NJ, b, D : D + 1], start=True, stop=True
                )
                op_ = pout.tile([128, D], mybir.dt.float32, tag="out")
                nc.tensor.matmul(
                    op_[:], qT[:NJ, :], hs_sb[:NJ, b, 0:D], start=True, stop=True
                )
                recip = softp.tile([128, 1], f32, tag="recip")
                nc.vector.reciprocal(recip[:], oq[:])
                out_sb = outsp.tile([128, D], f32, tag="osb")
                nc.vector.tensor_scalar_mul(out_sb[:], op_[:], recip[:])
                nc.sync.dma_start(out=out_d[b], in_=out_sb[:])
                if debug:
                    nc.sync.dma_start(out=dbg_vt[b], in_=VT[:])
                    nc.sync.dma_start(
                        out=dbg_st[b, :, :, :NJ], in_=ST[:, :, :NJ]
                    )
                    nc.sync.dma_start(
                        out=dbg_beta[b, :, :NJ], in_=beta_sb[:, :NJ]
                    )
                    nc.sync.dma_start(out=dbg_q[b, :NJ], in_=qT[:NJ, :])

    nc.compile()
    return nc


def _get_nc(nj: tuple):
    f_dve = float(os.environ.get("KERNEL_F_DVE", "0.55"))
    debug = os.environ.get("KERNEL_DEBUG", "0") == "1"
    key = (nj, f_dve, debug)
    if key not in _CACHE:
        _CACHE[key] = _build(nj, f_dve, debug)
    return _CACHE[key]


def _prep(h_s, h_v, lengths, W_S, b_S, W_V, b_V, W_w, b_w):
    h_s = np.asarray(h_s, dtype=np.float32)
    h_v = np.asarray(h_v, dtype=np.float32)
    lengths = np.asarray(lengths).astype(np.int64).reshape(B)
    # stable sort by descending length; deal rank 8j+c -> core c slot j
    perm = np.argsort(-lengths, kind="stable")
    nj = tuple(int(lengths[perm[8 * j]]) for j in range(BPC))
    mask = (
        lengths.reshape(B, 1) >= np.arange(1, N + 1).reshape(1, N)
    ).astype(np.float32)

    WS16 = np.ascontiguousarray(
        np.asarray(W_S, dtype=np.float32)
        .reshape(NC, 128, NC, 128)
        .transpose(1, 0, 2, 3),
        dtype=F16,
    )
    WV16 = np.ascontiguousarray(
        np.asarray(W_V, dtype=np.float32)
        .reshape(NC, 128, NC, 128)
        .transpose(1, 0, 2, 3),
        dtype=F16,
    )
    Ww16 = np.ascontiguousarray(
        np.asarray(W_w, dtype=np.float32).reshape(NC, 128).T, dtype=F16
    )
    bSV = np.ascontiguousarray(
        (np.asarray(b_S) + np.asarray(b_V)).astype(np.float32).reshape(NC, 128).T
    )
    bw = float(np.asarray(b_w).reshape(-1)[0])

    in_maps = []
    for c in range(NCORES):
        gidx = [int(perm[8 * j + c]) for j in range(BPC)]
        hvT = np.ascontiguousarray(
            h_v[gidx]
            .transpose(0, 2, 1)
            .reshape(BPC, NC, 128, T)
            .transpose(0, 2, 1, 3),
            dtype=F16,
        )
        hsT = np.ascontiguousarray(
            h_s[gidx]
            .transpose(0, 2, 1)
            .reshape(BPC, NC, 128, N)
            .transpose(0, 2, 1, 3),
            dtype=F16,
        )
        m_c = mask[gidx]  # (BPC, N)
        hs_aug = np.concatenate(
            [h_s[gidx] * m_c[:, :, None], m_c[:, :, None]], axis=2
        )  # (BPC, N, D+1)
        in_maps.append(
            {
                "hvT": hvT,
                "hsT": hsT,
                "hs_aug": np.ascontiguousarray(hs_aug, dtype=BF16),
                "WS16": WS16,
                "WV16": WV16,
                "Ww16": Ww16,
                "bSV": bSV,
                "mT": np.ascontiguousarray(m_c.T),  # (N, BPC)
                "bwmT": np.ascontiguousarray((bw * m_c).T),
            }
        )
    return nj, perm, in_maps


def run(inputs: dict, trace: bool = False):
    """Run on 8 NeuronCores; returns (output, BassKernelResults)."""
    from concourse import bass_utils

    nj, perm, in_maps = _prep(**inputs)
    nc = _get_nc(nj)
    res = bass_utils.run_bass_kernel_spmd(
        nc, in_maps, core_ids=list(range(NCORES)), trace=trace
    )
    full = np.empty((B, T, D), dtype=np.float32)
    for c in range(NCORES):
        o = res.results[c]["out"]
        for j in range(BPC):
            full[perm[8 * j + c]] = o[j]
    return full, res


def kernel(**inputs) -> np.ndarray:
    out, _ = run(inputs, trace=False)
    return out


# revision 20
# speedup vs baseline: 4.3313x; 1.0281x over previous
"""Trainium2 Bass kernel for InteractorwoLSTM additive attention.

out[b,t,:] = alpha[b,t,:] @ h_s[b]  with
  beta[b,t,n] = W_w . tanh(h_s[b,n]@W_S + b_S + h_v[b,t]@W_V + b_V) + b_w
  alpha = masked-softmax(beta) per reference semantics.

Sharding: data-parallel over batch B=32 across 8 cores (4 slots/core).
Batches are dealt to cores sorted by length (descending) so that slot j
has a compile-time bound NJ[j] = max length within slot j; all work in
the token dimension n is clipped to NJ[j] (masked-softmax semantics make
positions n >= length irrelevant except for an exp(0)=1 term that the
mask columns reproduce exactly).

Per core, per batch b (D=512 split into 4 chunks of 128 on partitions):
  PE  : VT = (h_v W_V)^T, ST = (h_s W_S)^T   (fp16 inputs, fp32 psum)
  ACT : VT psum -> sbuf copy
  DVE : ST psum -> sbuf copy, fusing the b_S+b_V bias (per-chunk)
  DVE/Pool: epre[d,n,t] = ST[d,n] + VT[d,t]  (split over n)
  ACT : e = tanh(epre) -> fp16
  PE  : beta[t,n] = sum_d e[d,n,t]*Ww[d]  (c-outer so chains interleave)
  PE  : betaT = transpose(beta)
  ACT : qT[n,t] = exp(betaT*mask[n] + b_w*mask[n]) -> bf16
  PE  : Qs = qT^T @ mask_col ; outraw = qT^T @ (h_s*mask)
  DVE : out = outraw * (1/Qs[t])

The tail (transpose/exp/final/scale) of batch b is issued interleaved
with the head of batch b+1 so no engine stalls on in-order waits.
"""

import os
import numpy as np
import ml_dtypes

B, T, N = 32, 128, 30
D = 512
NCORES = 8
BPC = B // NCORES  # batches per core (slots)
NC = D // 128  # 4 chunks

_CACHE = {}

F16 = np.float16
BF16 = ml_dtypes.bfloat16


def _build(nj: tuple, f_dve: float, debug: bool = False):
    """Compile the per-core program. nj[j] = token bound for slot j."""
    import concourse.bacc as bacc
    import concourse.tile as tile
    from concourse import mybir
    from concourse.masks import make_identity

    f32 = mybir.dt.float32
    f16 = mybir.dt.float16
    bf16 = mybir.dt.bfloat16
    Tanh = mybir.ActivationFunctionType.Tanh
    Exp = mybir.ActivationFunctionType.Exp
    Copy = mybir.ActivationFunctionType.Copy

    nc = bacc.Bacc(
        "TRN2",
        target_bir_lowering=False,
        debug=False,
        enable_asserts=True,
        num_devices=NCORES,
    )

    # ---- DRAM I/O (all pre-arranged/cast host-side) ----
    hvT_d = nc.dram_tensor("hvT", [BPC, 128, NC, 128], f16, kind="ExternalInput").ap()
    hsT_d = nc.dram_tensor("hsT", [BPC, 128, NC, N], f16, kind="ExternalInput").ap()
    hsa_d = nc.dram_tensor("hs_aug", [BPC, N, D + 1], bf16, kind="ExternalInput").ap()
    WS_d = nc.dram_tensor("WS16", [128, NC, NC, 128], f16, kind="ExternalInput").ap()
    WV_d = nc.dram_tensor("WV16", [128, NC, NC, 128], f16, kind="ExternalInput").ap()
    Ww_d = nc.dram_tensor("Ww16", [128, NC], f16, kind="ExternalInput").ap()
    bSV_d = nc.dram_tensor("bSV", [128, NC], f32, kind="ExternalInput").ap()
    mT_d = nc.dram_tensor("mT", [N, BPC], f32, kind="ExternalInput").ap()
    bwmT_d = nc.dram_tensor("bwmT", [N, BPC], f32, kind="ExternalInput").ap()
    out_d = nc.dram_tensor("out", [BPC, T, D], f32, kind="ExternalOutput").ap()
    if debug:
        dbg_vt = nc.dram_tensor(
            "dbg_vt", [BPC, 128, NC, 128], f32, kind="ExternalOutput"
        ).ap()
        dbg_st = nc.dram_tensor(
            "dbg_st", [BPC, 128, NC, N], f32, kind="ExternalOutput"
        ).ap()
        dbg_beta = nc.dram_tensor(
            "dbg_beta", [BPC, 128, N], f32, kind="ExternalOutput"
        ).ap()
        dbg_q = nc.dram_tensor(
            "dbg_q", [BPC, N, 128], mybir.dt.bfloat16, kind="ExternalOutput"
        ).ap()

    with tile.TileContext(nc) as tc:
        with (
            tc.tile_pool(name="const", bufs=1) as const,
            tc.tile_pool(name="hvt", bufs=2) as hvtp,
            tc.tile_pool(name="hst", bufs=2) as hstp,
            tc.tile_pool(name="proj", bufs=2) as projp,
            tc.tile_pool(name="epre", bufs=3) as eprep,
            tc.tile_pool(name="ebig", bufs=2) as ebigp,
            tc.tile_pool(name="soft", bufs=2) as softp,
            tc.tile_pool(name="outs", bufs=2) as outsp,
            tc.tile_pool(name="pvt", bufs=2, space="PSUM") as pvt,
            tc.tile_pool(name="pst", bufs=1, space="PSUM") as pst,
            tc.tile_pool(name="pbeta", bufs=1, space="PSUM") as pbeta,
            tc.tile_pool(name="pbt", bufs=1, space="PSUM") as pbt,
            tc.tile_pool(name="pout", bufs=2, space="PSUM") as pout,
        ):
            # ---- constants / weights ----
            ident = const.tile([128, 128], f32)
            make_identity(nc, ident[:])
            WS_sb = const.tile([128, NC, NC, 128], f16)
            nc.sync.dma_start(out=WS_sb[:], in_=WS_d)
            WV_sb = const.tile([128, NC, NC, 128], f16)
            nc.sync.dma_start(out=WV_sb[:], in_=WV_d)
            Ww_sb = const.tile([128, NC], f16)
            nc.sync.dma_start(out=Ww_sb[:], in_=Ww_d)
            bSV_sb = const.tile([128, NC], f32)
            nc.sync.dma_start(out=bSV_sb[:], in_=bSV_d)
            mT_sb = const.tile([N, BPC], f32)
            nc.sync.dma_start(out=mT_sb[:], in_=mT_d)
            bwmT_sb = const.tile([N, BPC], f32)
            nc.sync.dma_start(out=bwmT_sb[:], in_=bwmT_d)
            hs_sb = const.tile([N, BPC, D + 1], bf16)
            for b in range(BPC):
                nc.sync.dma_start(out=hs_sb[:, b, :], in_=hsa_d[b])

            prev = None  # stage-A tiles of batch b-1 awaiting their tail

            def tail_head(pv):
                """Transpose -> exp -> final matmuls for a previous batch.
                Issued before the current batch's tanh/beta so neither the
                ACT nor the PE stream stalls in-order on late deps."""
                b, NJ, beta_sb = pv
                bt = pbt.tile([32, 128], f32, tag="bt")
                nc.tensor.transpose(bt[:NJ, :], beta_sb[:, :NJ], ident[:])
                qT = softp.tile([N, 128], bf16, tag="qT")
                nc.scalar.activation(
                    qT[:NJ, :],
                    bt[:NJ, :],
                    Exp,
                    bias=bwmT_sb[:NJ, b : b + 1],
                    scale=mT_sb[:NJ, b : b + 1],
                )
                oq = pbt.tile([128, 1], f32, tag="oq")
                nc.tensor.matmul(
                    oq[:], qT[:NJ, :], hs_sb[:NJ, b, D : D + 1], start=True, stop=True
                )
                op_ = pout.tile([128, D], f32, tag="out")
                nc.tensor.matmul(
                    op_[:], qT[:NJ, :], hs_sb[:NJ, b, 0:D], start=True, stop=True
                )
                if debug:
                    nc.sync.dma_start(out=dbg_q[b, :NJ], in_=qT[:NJ, :])
                return b, oq, op_

            def tail_tail(pv2):
                """Recip + scale + out DMA; issued after the current batch's
                adds so the DVE stream isn't blocked ahead of them."""
                b, oq, op_ = pv2
                recip = softp.tile([128, 1], f32, tag="recip")
                nc.vector.reciprocal(recip[:], oq[:])
                out_sb = outsp.tile([128, D], f32, tag="osb")
                nc.vector.tensor_scalar_mul(out_sb[:], op_[:], recip[:])
                nc.sync.dma_start(out=out_d[b], in_=out_sb[:])

            for b in range(BPC):
                NJ = nj[b]
                # ---- load pre-transposed activations ----
                hvT = hvtp.tile([128, NC, 128], f16, tag="hvT")
                nc.sync.dma_start(out=hvT[:], in_=hvT_d[b])
                hsT = hstp.tile([128, NC, N], f16, tag="hsT")
                nc.sync.dma_start(out=hsT[:], in_=hsT_d[b])

                # ---- projections (kc-outer: consecutive matmuls hit
                # different psum columns, so their latencies overlap) ----
                ps_v = pvt.tile([128, D], f32, tag="vt")
                for kc in range(NC):
                    for mc in range(NC):
                        nc.tensor.matmul(
                            ps_v[:, mc * 128 : (mc + 1) * 128],
                            WV_sb[:, kc, mc, :],
                            hvT[:, kc, :],
                            start=(kc == 0 and mc == 0),
                            stop=(kc == NC - 1 and mc == NC - 1),
                        )
                ps_s = pst.tile([128, NC, N], f32, tag="st")
                for kc in range(NC):
                    for mc in range(NC):
                        nc.tensor.matmul(
                            ps_s[:, mc, :NJ],
                            WS_sb[:, kc, mc, :],
                            hsT[:, kc, :NJ],
                            start=(kc == 0 and mc == 0),
                            stop=(kc == NC - 1 and mc == NC - 1),
                        )

                # copies out of psum: VT on ACT (one op), ST on DVE (bias)
                VT = projp.tile([128, NC, 128], f32, tag="VT")
                nc.scalar.activation(VT[:], ps_v[:], Copy)
                ST = projp.tile([128, NC, N], f32, tag="ST")
                for mc in range(NC):
                    nc.vector.tensor_scalar_add(
                        ST[:, mc, :NJ], ps_s[:, mc, :NJ], bSV_sb[:, mc : mc + 1]
                    )

                # ---- previous batch's tail (part 1): transpose/exp/finals
                # land before this batch's tanh (ACT) and beta (PE) ----
                ptail = tail_head(prev) if prev is not None else None

                # ---- e = tanh(ST + VT); DVE/Pool split over n ----
                k = min(NJ, max(1, int(round(NJ * f_dve))))
                ebig = ebigp.tile([128, NC, N, 128], f16, tag="e")
                for c in range(NC):
                    epre = eprep.tile([128, N, 128], f32, tag="epre")
                    vt_b = VT[:, c, :].unsqueeze(1)
                    nc.vector.tensor_add(
                        epre[:, :k, :],
                        ST[:, c, :k].unsqueeze(2).broadcast_to([128, k, 128]),
                        vt_b.broadcast_to([128, k, 128]),
                    )
                    if k < NJ:
                        nc.gpsimd.tensor_add(
                            epre[:, k:NJ, :],
                            ST[:, c, k:NJ].unsqueeze(2).broadcast_to(
                                [128, NJ - k, 128]
                            ),
                            vt_b.broadcast_to([128, NJ - k, 128]),
                        )
                    nc.scalar.activation(
                        ebig[:, c, :NJ, :], epre[:, :NJ, :], Tanh
                    )

                # ---- previous batch's tail (part 2): recip/scale/dma ----
                if ptail is not None:
                    tail_tail(ptail)

                # ---- beta: c-outer so consecutive matmuls are independent
                # (different psum columns) and chunk c streams while the
                # ACT is still tanh-ing chunk c+1 ----
                bp = pbeta.tile([128, N], f32, tag="beta")
                for c in range(NC):
                    for n in range(NJ):
                        nc.tensor.matmul(
                            bp[:, n : n + 1],
                            ebig[:, c, n, :],
                            Ww_sb[:, c : c + 1],
                            start=(c == 0 and n == 0),
                            stop=(c == NC - 1 and n == NJ - 1),
                        )
                beta_sb = softp.tile([128, N], f32, tag="bsb")
                nc.vector.tensor_copy(beta_sb[:, :NJ], bp[:, :NJ])
                if debug:
                    nc.sync.dma_start(out=dbg_vt[b], in_=VT[:])
                    nc.sync.dma_start(out=dbg_st[b, :, :, :NJ], in_=ST[:, :, :NJ])
                    nc.sync.dma_start(out=dbg_beta[b, :, :NJ], in_=beta_sb[:, :NJ])

                prev = (b, NJ, beta_sb)

            tail_tail(tail_head(prev))

    nc.compile()
    return nc


def _get_nc(nj: tuple):
    f_dve = float(os.environ.get("KERNEL_F_DVE", "0.66"))
    debug = os.environ.get("KERNEL_DEBUG", "0") == "1"
    key = (nj, f_dve, debug)
    if key not in _CACHE:
        _CACHE[key] = _build(nj, f_dve, debug)
    return _CACHE[key]


def _prep(h_s, h_v, lengths, W_S, b_S, W_V, b_V, W_w, b_w):
    h_s = np.asarray(h_s, dtype=np.float32)
    h_v = np.asarray(h_v, dtype=np.float32)
    lengths = np.asarray(lengths).astype(np.int64).reshape(B)
    # stable sort by descending length; deal rank 8j+c -> core c slot j
    perm = np.argsort(-lengths, kind="stable")
    nj = tuple(int(lengths[perm[8 * j]]) for j in range(BPC))
    mask = (
        lengths.reshape(B, 1) >= np.arange(1, N + 1).reshape(1, N)
    ).astype(np.float32)

    WS16 = np.ascontiguousarray(
        np.asarray(W_S, dtype=np.float32)
        .reshape(NC, 128, NC, 128)
        .transpose(1, 0, 2, 3),
        dtype=F16,
    )
    WV16 = np.ascontiguousarray(
        np.asarray(W_V, dtype=np.float32)
        .reshape(NC, 128, NC, 128)
        .transpose(1, 0, 2, 3),
        dtype=F16,
    )
    Ww16 = np.ascontiguousarray(
        np.asarray(W_w, dtype=np.float32).reshape(NC, 128).T, dtype=F16
    )
    bSV = np.ascontiguousarray(
        (np.asarray(b_S) + np.asarray(b_V)).astype(np.float32).reshape(NC, 128).T
    )
    bw = float(np.asarray(b_w).reshape(-1)[0])

    in_maps = []
    for c in range(NCORES):
        gidx = [int(perm[8 * j + c]) for j in range(BPC)]
        hvT = np.ascontiguousarray(
            h_v[gidx]
            .transpose(0, 2, 1)
            .reshape(BPC, NC, 128, T)
            .transpose(0, 2, 1, 3),
            dtype=F16,
        )
        hsT = np.ascontiguousarray(
            h_s[gidx]
            .transpose(0, 2, 1)
            .reshape(BPC, NC, 128, N)
            .transpose(0, 2, 1, 3),
            dtype=F16,
        )
        m_c = mask[gidx]  # (BPC, N)
        hs_aug = np.concatenate(
            [h_s[gidx] * m_c[:, :, None], m_c[:, :, None]], axis=2
        )  # (BPC, N, D+1)
        in_maps.append(
            {
                "hvT": hvT,
                "hsT": hsT,
                "hs_aug": np.ascontiguousarray(hs_aug, dtype=BF16),
                "WS16": WS16,
                "WV16": WV16,
                "Ww16": Ww16,
                "bSV": bSV,
                "mT": np.ascontiguousarray(m_c.T),  # (N, BPC)
                "bwmT": np.ascontiguousarray((bw * m_c).T),
            }
        )
    return nj, perm, in_maps


def run(inputs: dict, trace: bool = False):
    """Run on 8 NeuronCores; returns (output, BassKernelResults)."""
    from concourse import bass_utils

    nj, perm, in_maps = _prep(**inputs)
    nc = _get_nc(nj)
    res = bass_utils.run_bass_kernel_spmd(
        nc, in_maps, core_ids=list(range(NCORES)), trace=trace
    )
    full = np.empty((B, T, D), dtype=np.float32)
    for c in range(NCORES):
        o = res.results[c]["out"]
        for j in range(BPC):
            full[perm[8 * j + c]] = o[j]
    return full, res


def kernel(**inputs) -> np.ndarray:
    out, _ = run(inputs, trace=False)
    return out
